# revision 21
# baseline (speedup 1.0000x reference)
"""Single-head causal attention (B=8, T=2048, E=H=1024) on 8 TRN2 NeuronCores.

Strategy: data-parallel over batch (one batch element per core). Per core:
  phase A1: qT = (x@Wq.T).T and kT = (x@Wk.T).T   [H,T], contraction over E.
            Both need x.T as the moving operand, so the host pre-transposes
            x (and the weights) before upload. qT spills to DRAM; kT stays
            resident in SBUF. x.T is resident during A1; Wq.T/Wk.T stream
            as [128,128] stationary blocks.
  phase A2: v = x@Wv.T  [T,H]  (x.T blocks stationary, Wv.T moving),
            resident in SBUF. Wv.T is preloaded during A1 so A2 starts
            with no DMA stall.
  phase B:  causal flash attention over t-chunks of 256, computed in the
            S^T orientation: S^T[s,t] = sum_h kT[h,s]*qT[h,t], so softmax
            weights come out as wT[s_block, t] tiles that feed the output
            matmul O[t,h] += wT.T @ v[s_block] directly — no transposes
            anywhere. Row sums ride along as N=2 matmuls against a ones
            column; normalization is a reciprocal + tensor_scalar multiply
            at the end. exp() needs no max-subtraction: scores are ~N(0,1)
            so exp never overflows fp32.

All matmuls run in float32r (TRN2's full-rate ~12-bit-significand fp32 mode;
~4e-4 end-to-end vs the fp32 reference) with fp32 PSUM accumulation.
"""

import numpy as np

import concourse.bacc as bacc
import concourse.mybir as mybir
import concourse.tile as tile
from concourse.bass_utils import run_bass_kernel_spmd

B, T, E, H = 8, 2048, 1024, 1024
N_CORES = 8
SCALE = float(E) ** -0.5

DT = mybir.dt.float32r
F32 = mybir.dt.float32

TCB = 256            # phase-B t-chunk width
N_TCB = T // TCB     # 8
N_EB = E // 128      # 8  e-blocks
N_HB = H // 128      # 8  h-blocks
N_SB = T // 128      # 16 s-blocks


def build_program():
    nc = bacc.Bacc("TRN2", target_bir_lowering=False, debug=False,
                   num_devices=N_CORES)

    # host-prepped layouts: every DMA reads long contiguous runs
    xT_d = nc.declare_dram_parameter("xA", [4, 128, N_EB, 512], DT,
                                     isOutput=False)   # [t4][p][ek][t]
    xV_d = nc.declare_dram_parameter("xV", [8, 128, N_EB, 256], DT,
                                     isOutput=False)   # [t8][p][ek][t]
    wqT_d = nc.declare_dram_parameter("WqT", [N_HB, 128, N_EB, 128], DT,
                                      isOutput=False)  # [hm][p][ek][h]
    wkT_d = nc.declare_dram_parameter("WkT", [N_HB, 128, N_EB, 128], DT,
                                      isOutput=False)
    wvT_d = nc.declare_dram_parameter("WvT", [E, H], DT, isOutput=False)
    out_d = nc.declare_dram_parameter("out", [T, H], F32, isOutput=True)

    qT_dram = nc.dram_tensor("qT_spill", [H, T], DT)

    with tile.TileContext(nc) as tc:
        with (
            tc.tile_pool(name="kt", bufs=1) as pool_kt,
            tc.tile_pool(name="misc", bufs=1) as pool_misc,
            tc.tile_pool(name="wv", bufs=1) as pool_wv,
        ):
            kt = [pool_kt.tile([128, T], DT, tag=f"kt{k}", name=f"kt{k}")
                  for k in range(N_HB)]
            ones_f = pool_misc.tile([128, 1], F32, tag="ones_f", name="ones_f")
            ones = pool_misc.tile([128, 1], DT, tag="ones", name="ones")
            nc.gpsimd.memset(ones_f[:], 1.0)
            nc.vector.tensor_copy(ones[:], ones_f[:])

            # first A2 x-chunk, prefetched from the start (avoids the
            # phase-boundary stall on SBUF address reuse)
            xv_pre = pool_misc.tile([128, N_EB, 256], DT, tag="xv_pre",
                                    name="xv_pre")

            # Wv.T halves, preloaded early: wvh[ek][hc] = WvT[ek*128:+128, hc*512:+512]
            wvh = [[pool_wv.tile([128, 512], DT, tag=f"wv{k}_{hc}",
                                 name=f"wv{k}_{hc}") for hc in range(2)]
                   for k in range(N_EB)]

            # ---------------- phase A1: qT (spilled) + kT (resident) -------
            with (
                tc.tile_pool(name="xf", bufs=1) as pool_xf,
                tc.tile_pool(name="wqk", bufs=2) as pool_wqk,
                tc.tile_pool(name="qs", bufs=2) as pool_qs,
                tc.tile_pool(name="pa", bufs=3, space="PSUM") as psum_a,
            ):
                # x.T fully resident: one [128, 8, 512] tile per t4-chunk.
                # Chunk 0 is split in half so the very first matmuls only
                # wait on 1MB of x instead of 2MB.
                xf0a = pool_xf.tile([128, N_EB // 2, 512], DT, tag="xf0a",
                                    name="xf0a")
                xf0b = pool_xf.tile([128, N_EB // 2, 512], DT, tag="xf0b",
                                    name="xf0b")
                xf = [None] + [pool_xf.tile([128, N_EB, 512], DT,
                                            tag=f"xf{t4}", name=f"xf{t4}")
                               for t4 in range(1, 4)]

                def xf_slice(t4, ek):
                    if t4 == 0:
                        t = xf0a if ek < N_EB // 2 else xf0b
                        return t[:, ek % (N_EB // 2), :]
                    return xf[t4][:, ek, :]

                def load_xf(t4):
                    # chunk 0 on the sync queue (critical path with W0);
                    # chunks 1-3 on the scalar HWDGE queue, which is idle
                    # until the first qs spill -- the two queues run the
                    # A1 prefetch in parallel.
                    if t4 == 0:
                        half = N_EB // 2
                        nc.sync.dma_start(xf0a[:], xT_d[0, :, 0:half, :])
                        nc.sync.dma_start(xf0b[:], xT_d[0, :, half:N_EB, :])
                        return
                    nc.scalar.dma_start(xf[t4][:], xT_d[t4, :, :, :])

                def alloc_load_w(hm):
                    wqb = pool_wqk.tile([128, N_EB, 128], DT, tag="wqb",
                                        name=f"wqb{hm}")
                    wkb = pool_wqk.tile([128, N_EB, 128], DT, tag="wkb",
                                        name=f"wkb{hm}")
                    nc.sync.dma_start(wqb[:], wqT_d[hm, :, :, :])
                    nc.sync.dma_start(wkb[:], wkT_d[hm, :, :, :])
                    return wqb, wkb

                # consumption-ordered: x chunk 0, W for hm 0, x chunk 1,
                # W for hm 1, rest of x. wqk bufs=2 bounds W prefetch to
                # two hm's; later hm's W loads are emitted in the loop.
                w_pref = {0: alloc_load_w(0)}
                load_xf(0)
                load_xf(1)
                w_pref[1] = alloc_load_w(1)
                load_xf(2)
                load_xf(3)
                nc.scalar.dma_start(xv_pre[:], xV_d[0, :, :, :])

                with nc.named_scope("proj_qk"):
                    for hm in range(N_HB):
                        if hm in w_pref:
                            wqb, wkb = w_pref.pop(hm)
                        else:
                            wqb, wkb = alloc_load_w(hm)
                        if hm == 4:
                            # Wv preload for phase A2, timed mid-A1
                            for hc in range(2):
                                for k in range(N_EB):
                                    nc.scalar.dma_start(
                                        wvh[k][hc][:],
                                        wvT_d[k * 128:(k + 1) * 128,
                                              hc * 512:(hc + 1) * 512])
                        for t4 in range(4):
                            pq = psum_a.tile([128, 512], F32, tag="pq",
                                             name=f"pq_{hm}_{t4}")
                            pk = psum_a.tile([128, 512], F32, tag="pk",
                                             name=f"pk_{hm}_{t4}")
                            for ek in range(N_EB):
                                nc.tensor.matmul(
                                    pq[:], wqb[:, ek, :], xf_slice(t4, ek),
                                    start=(ek == 0), stop=(ek == N_EB - 1))
                            for ek in range(N_EB):
                                nc.tensor.matmul(
                                    pk[:], wkb[:, ek, :], xf_slice(t4, ek),
                                    start=(ek == 0), stop=(ek == N_EB - 1))
                            qs = pool_qs.tile([128, 512], DT, tag="qs",
                                              name=f"qs_{hm}_{t4}")
                            nc.scalar.copy(qs[:], pq[:])
                            nc.scalar.dma_start(
                                qT_dram[hm * 128:(hm + 1) * 128,
                                        t4 * 512:(t4 + 1) * 512], qs[:])
                            nc.vector.tensor_copy(
                                kt[hm][:, t4 * 512:(t4 + 1) * 512], pk[:])

            with (
                tc.tile_pool(name="v", bufs=1) as pool_v,
                tc.tile_pool(name="qpre", bufs=1) as pool_qpre,
            ):
                vt = [pool_v.tile([128, H], DT, tag=f"v{j}", name=f"v{j}")
                      for j in range(N_SB)]
                qc_pre = pool_qpre.tile([128, N_HB, TCB], DT, tag="qc_pre",
                                        name="qc_pre")
                nc.sync.dma_start(
                    qc_pre[:],
                    qT_dram[:, 0:TCB].rearrange("(k p) t -> p k t", p=128))

                # ---------------- phase A2: v (resident) -------------------
                with (
                    tc.tile_pool(name="x2", bufs=2) as pool_x2,
                    tc.tile_pool(name="pv", bufs=4, space="PSUM") as psum_v,
                ):
                    with nc.named_scope("proj_v"):
                        for t8 in range(T // 256):
                            if t8 == 0:
                                xv = xv_pre
                            else:
                                xv = pool_x2.tile([128, N_EB, 256], DT,
                                                  tag="xv", name=f"xv_{t8}")
                                nc.sync.dma_start(xv[:], xV_d[t8, :, :, :])
                            for ss in range(2):
                                j = t8 * 2 + ss
                                for hc in range(2):
                                    pv = psum_v.tile([128, 512], F32, tag="pv",
                                                     name=f"pv_{t8}_{ss}_{hc}")
                                    for ek in range(N_EB):
                                        nc.tensor.matmul(
                                            pv[:],
                                            xv[:, ek, ss * 128:(ss + 1) * 128],
                                            wvh[ek][hc][:],
                                            start=(ek == 0), stop=(ek == N_EB - 1))
                                    if hc == 0:
                                        nc.vector.tensor_copy(
                                            vt[j][:, hc * 512:(hc + 1) * 512], pv[:])
                                    else:
                                        nc.scalar.copy(
                                            vt[j][:, hc * 512:(hc + 1) * 512], pv[:])

                # ---------------- phase B: causal attention ----------------
                with (
                    tc.tile_pool(name="q", bufs=2) as pool_q,
                    tc.tile_pool(name="wt", bufs=3) as pool_wt,
                    tc.tile_pool(name="ob", bufs=4) as pool_ob,
                    tc.tile_pool(name="sm", bufs=4) as pool_sm,
                    tc.tile_pool(name="pb", bufs=1, space="PSUM") as psum_b,
                ):
                    with nc.named_scope("attn"):
                        for c in range(N_TCB):
                            if c == 0:
                                qc = qc_pre
                            else:
                                qc = pool_q.tile([128, N_HB, TCB], DT,
                                                 tag="qc", name=f"qc_{c}")
                                nc.sync.dma_start(
                                    qc[:],
                                    qT_dram[:, c * TCB:(c + 1) * TCB]
                                    .rearrange("(k p) t -> p k t", p=128))
                            n_j = 2 * c + 2
                            o_ps = [psum_b.tile([128, 512], F32, tag=f"O{i}",
                                                name=f"O_{c}_{i}")
                                    for i in range(4)]
                            rs_ps = psum_b.tile([1, TCB], F32, tag="rs",
                                                name=f"rs_{c}")

                            def scores(j, c=c, qc=qc):
                                s_ps = psum_b.tile([128, TCB], F32,
                                                   tag=f"S{j % 2}",
                                                   name=f"S_{c}_{j}")
                                for hk in range(N_HB):
                                    nc.tensor.matmul(
                                        s_ps[:],
                                        kt[hk][:, j * 128:(j + 1) * 128],
                                        qc[:, hk, :],
                                        start=(hk == 0), stop=(hk == N_HB - 1))
                                wt = pool_wt.tile([128, TCB], DT, tag="wt",
                                                  name=f"wt_{c}_{j}")
                                nc.scalar.activation(
                                    wt[:], s_ps[:],
                                    mybir.ActivationFunctionType.Exp,
                                    scale=SCALE)
                                if j >= 2 * c:
                                    # zero the strictly-upper (s > t) part
                                    nc.gpsimd.affine_select(
                                        out=wt[:], in_=wt[:],
                                        compare_op=mybir.AluOpType.is_ge,
                                        fill=0.0,
                                        base=c * TCB - j * 128,
                                        channel_multiplier=-1,
                                        pattern=[[1, TCB]])
                                return wt

                            def o_accum(j, wt, c=c, n_j=n_j, o_ps=o_ps,
                                        rs_ps=rs_ps):
                                first, last = (j == 0), (j == n_j - 1)
                                nc.tensor.matmul(
                                    rs_ps[:], ones[:], wt[:],
                                    start=first, stop=last)
                                for ts in range(2):
                                    if ts == 0 and j == n_j - 1 and n_j >= 2:
                                        # s-block 2c+1 is fully masked for
                                        # t-half 0: all-zero contribution
                                        continue
                                    wslice = wt[:, ts * 128:(ts + 1) * 128]
                                    last_ts = (j == n_j - 2) if ts == 0 \
                                        else last
                                    for hc in range(2):
                                        nc.tensor.matmul(
                                            o_ps[ts * 2 + hc][:], wslice,
                                            vt[j][:, hc * 512:(hc + 1) * 512],
                                            start=first, stop=last_ts)

                            # software pipeline: scores(j+1) issued ahead of
                            # O(j) so the PE never waits on the exp chain
                            wt_cur = scores(0)
                            for j in range(n_j):
                                wt_next = scores(j + 1) if j + 1 < n_j else None
                                o_accum(j, wt_cur)
                                wt_cur = wt_next
                            rs_sb = pool_sm.tile([1, TCB], F32, tag="rs_sb",
                                                 name=f"rs_sb_{c}")
                            nc.vector.tensor_copy(rs_sb[:], rs_ps[:])
                            for ts in range(2):
                                # transpose [1,128] -> [128,1] via K=1 matmul
                                rs_col = psum_b.tile([128, 1], F32,
                                                     tag="rs_col",
                                                     name=f"rs_col_{c}_{ts}")
                                nc.tensor.matmul(
                                    rs_col[:],
                                    rs_sb[0:1, ts * 128:(ts + 1) * 128],
                                    ones_f[0:1, 0:1],
                                    start=True, stop=True)
                                rec = pool_sm.tile([128, 1], F32, tag="rec",
                                                   name=f"rec_{c}_{ts}")
                                nc.vector.reciprocal(rec[:], rs_col[:])
                                for hc in range(2):
                                    ob = pool_ob.tile([128, 512], F32, tag="ob",
                                                      name=f"ob_{c}_{ts}_{hc}")
                                    if hc == 0:
                                        nc.vector.tensor_scalar_mul(
                                            ob[:], o_ps[ts * 2 + hc][:], rec[:])
                                    else:
                                        nc.scalar.activation(
                                            ob[:], o_ps[ts * 2 + hc][:],
                                            mybir.ActivationFunctionType.Copy,
                                            scale=rec[:])
                                    nc.gpsimd.dma_start(
                                        out_d[c * TCB + ts * 128:
                                              c * TCB + (ts + 1) * 128,
                                              hc * 512:(hc + 1) * 512], ob[:])

    nc.compile()
    return nc


_NC_CACHE = None


def _get_program():
    global _NC_CACHE
    if _NC_CACHE is None:
        _NC_CACHE = build_program()
    return _NC_CACHE


def make_in_maps(x, Wk, Wq, Wv):
    x = np.asarray(x, np.float32)
    xT = np.transpose(x, (0, 2, 1))                        # [B, E, T]
    # A1 layout [t4][p][ek][512]: xT[e, t] with e = ek*128 + p
    xA = np.ascontiguousarray(
        xT.reshape(B, N_EB, 128, 4, 512).transpose(0, 3, 2, 1, 4))
    # A2 layout [t8][p][ek][256]
    xV = np.ascontiguousarray(
        xT.reshape(B, N_EB, 128, 8, 256).transpose(0, 3, 2, 1, 4))

    def prep_w(W):   # [H,E] -> W.T [E,H] -> [hm][p][ek][128]
        WT = np.asarray(W, np.float32).T
        return np.ascontiguousarray(
            WT.reshape(N_EB, 128, N_HB, 128).transpose(2, 1, 0, 3))

    WqT = prep_w(Wq)
    WkT = prep_w(Wk)
    WvT = np.ascontiguousarray(np.asarray(Wv, np.float32).T)  # [E, H]
    return [{"xA": xA[b], "xV": xV[b], "WqT": WqT, "WkT": WkT, "WvT": WvT}
            for b in range(B)]


def kernel(x, Wk, Wq, Wv, _trace=False, _tmpdir=None):
    nc = _get_program()
    in_maps = make_in_maps(x, Wk, Wq, Wv)
    res = run_bass_kernel_spmd(nc, in_maps, list(range(N_CORES)),
                               trace=_trace, tmpdir=_tmpdir)
    out = np.stack([res.results[b]["out"] for b in range(B)])
    if _trace:
        kernel.last_result = res
    return out


# revision 22
# speedup vs baseline: 1.0040x; 1.0040x over previous
"""Single-head causal attention (B=8, T=2048, E=H=1024) on 8 TRN2 NeuronCores.

Strategy: data-parallel over batch (one batch element per core). Per core:
  phase A1: qT = (x@Wq.T).T and kT = (x@Wk.T).T   [H,T], contraction over E.
            Both need x.T as the moving operand, so the host pre-transposes
            x (and the weights) before upload. qT spills to DRAM; kT stays
            resident in SBUF. x.T is resident during A1; Wq.T/Wk.T stream
            as [128,128] stationary blocks.
  phase A2: v = x@Wv.T  [T,H]  (x.T blocks stationary, Wv.T moving),
            resident in SBUF. Wv.T is preloaded during A1 so A2 starts
            with no DMA stall.
  phase B:  causal flash attention over t-chunks of 256, computed in the
            S^T orientation: S^T[s,t] = sum_h kT[h,s]*qT[h,t], so softmax
            weights come out as wT[s_block, t] tiles that feed the output
            matmul O[t,h] += wT.T @ v[s_block] directly — no transposes
            anywhere. Row sums ride along as N=2 matmuls against a ones
            column; normalization is a reciprocal + tensor_scalar multiply
            at the end. exp() needs no max-subtraction: scores are ~N(0,1)
            so exp never overflows fp32.

All matmuls run in float32r (TRN2's full-rate ~12-bit-significand fp32 mode;
~4e-4 end-to-end vs the fp32 reference) with fp32 PSUM accumulation.
"""

import numpy as np

import concourse.bacc as bacc
import concourse.mybir as mybir
import concourse.tile as tile
from concourse.bass_utils import run_bass_kernel_spmd

B, T, E, H = 8, 2048, 1024, 1024
N_CORES = 8
SCALE = float(E) ** -0.5

DT = mybir.dt.float32r
F32 = mybir.dt.float32

TCB = 256            # phase-B t-chunk width
N_TCB = T // TCB     # 8
N_EB = E // 128      # 8  e-blocks
N_HB = H // 128      # 8  h-blocks
N_SB = T // 128      # 16 s-blocks


def build_program():
    nc = bacc.Bacc("TRN2", target_bir_lowering=False, debug=False,
                   num_devices=N_CORES)

    # host-prepped layouts: every DMA reads long contiguous runs
    xT_d = nc.declare_dram_parameter("xA", [4, 128, N_EB, 512], DT,
                                     isOutput=False)   # [t4][p][ek][t]
    xV_d = nc.declare_dram_parameter("xV", [8, 128, N_EB, 256], DT,
                                     isOutput=False)   # [t8][p][ek][t]
    wqT_d = nc.declare_dram_parameter("WqT", [N_HB, 128, N_EB, 128], DT,
                                      isOutput=False)  # [hm][p][ek][h]
    wkT_d = nc.declare_dram_parameter("WkT", [N_HB, 128, N_EB, 128], DT,
                                      isOutput=False)
    wvT_d = nc.declare_dram_parameter("WvT", [E, H], DT, isOutput=False)
    out_d = nc.declare_dram_parameter("out", [T, H], F32, isOutput=True)

    qT_dram = nc.dram_tensor("qT_spill", [H, T], DT)

    with tile.TileContext(nc) as tc:
        with (
            tc.tile_pool(name="kt", bufs=1) as pool_kt,
            tc.tile_pool(name="misc", bufs=1) as pool_misc,
            tc.tile_pool(name="wv", bufs=1) as pool_wv,
        ):
            kt = [pool_kt.tile([128, T], DT, tag=f"kt{k}", name=f"kt{k}")
                  for k in range(N_HB)]
            ones_f = pool_misc.tile([128, 1], F32, tag="ones_f", name="ones_f")
            ones = pool_misc.tile([128, 1], DT, tag="ones", name="ones")
            nc.gpsimd.memset(ones_f[:], 1.0)
            nc.vector.tensor_copy(ones[:], ones_f[:])

            # first A2 x-chunk, prefetched from the start (avoids the
            # phase-boundary stall on SBUF address reuse)
            xv_pre = pool_misc.tile([128, N_EB, 256], DT, tag="xv_pre",
                                    name="xv_pre")

            # Wv.T halves, preloaded early: wvh[ek][hc] = WvT[ek*128:+128, hc*512:+512]
            wvh = [[pool_wv.tile([128, 512], DT, tag=f"wv{k}_{hc}",
                                 name=f"wv{k}_{hc}") for hc in range(2)]
                   for k in range(N_EB)]

            # ---------------- phase A1: qT (spilled) + kT (resident) -------
            with (
                tc.tile_pool(name="xf", bufs=1) as pool_xf,
                tc.tile_pool(name="wqk", bufs=2) as pool_wqk,
                tc.tile_pool(name="qs", bufs=2) as pool_qs,
                tc.tile_pool(name="pa", bufs=3, space="PSUM") as psum_a,
            ):
                # x.T fully resident: one [128, 8, 512] tile per t4-chunk.
                # Chunk 0 is split in half so the very first matmuls only
                # wait on 1MB of x instead of 2MB.
                xf0a = pool_xf.tile([128, N_EB // 2, 512], DT, tag="xf0a",
                                    name="xf0a")
                xf0b = pool_xf.tile([128, N_EB // 2, 512], DT, tag="xf0b",
                                    name="xf0b")
                xf = [None] + [pool_xf.tile([128, N_EB, 512], DT,
                                            tag=f"xf{t4}", name=f"xf{t4}")
                               for t4 in range(1, 4)]

                def xf_slice(t4, ek):
                    if t4 == 0:
                        t = xf0a if ek < N_EB // 2 else xf0b
                        return t[:, ek % (N_EB // 2), :]
                    return xf[t4][:, ek, :]

                def load_xf(t4):
                    # chunk 0 on the sync queue (critical path with W0);
                    # chunks 1-3 on the scalar HWDGE queue, which is idle
                    # until the first qs spill -- the two queues run the
                    # A1 prefetch in parallel.
                    if t4 == 0:
                        half = N_EB // 2
                        nc.sync.dma_start(xf0a[:], xT_d[0, :, 0:half, :])
                        nc.sync.dma_start(xf0b[:], xT_d[0, :, half:N_EB, :])
                        return
                    nc.scalar.dma_start(xf[t4][:], xT_d[t4, :, :, :])

                def alloc_load_w(hm):
                    wqb = pool_wqk.tile([128, N_EB, 128], DT, tag="wqb",
                                        name=f"wqb{hm}")
                    wkb = pool_wqk.tile([128, N_EB, 128], DT, tag="wkb",
                                        name=f"wkb{hm}")
                    nc.sync.dma_start(wqb[:], wqT_d[hm, :, :, :])
                    nc.sync.dma_start(wkb[:], wkT_d[hm, :, :, :])
                    return wqb, wkb

                # consumption-ordered: x chunk 0, W for hm 0, x chunk 1,
                # W for hm 1, rest of x. wqk bufs=2 bounds W prefetch to
                # two hm's; later hm's W loads are emitted in the loop.
                w_pref = {0: alloc_load_w(0)}
                load_xf(0)
                # gate the scalar-queue bulk loads behind the arrival of the
                # first x chunk: the ACT sequencer blocks on this copy, so
                # the critical sync-queue loads get full HBM bandwidth first
                gate = pool_misc.tile([1, 1], DT, tag="gate", name="gate")
                nc.scalar.copy(gate[:], xf0a[0:1, 0, 0:1])
                load_xf(1)
                w_pref[1] = alloc_load_w(1)
                load_xf(2)
                load_xf(3)
                nc.scalar.dma_start(xv_pre[:], xV_d[0, :, :, :])

                with nc.named_scope("proj_qk"):
                    for hm in range(N_HB):
                        if hm in w_pref:
                            wqb, wkb = w_pref.pop(hm)
                        else:
                            wqb, wkb = alloc_load_w(hm)
                        if hm == 4:
                            # Wv preload for phase A2, timed mid-A1
                            for hc in range(2):
                                for k in range(N_EB):
                                    nc.scalar.dma_start(
                                        wvh[k][hc][:],
                                        wvT_d[k * 128:(k + 1) * 128,
                                              hc * 512:(hc + 1) * 512])
                        for t4 in range(4):
                            pq = psum_a.tile([128, 512], F32, tag="pq",
                                             name=f"pq_{hm}_{t4}")
                            pk = psum_a.tile([128, 512], F32, tag="pk",
                                             name=f"pk_{hm}_{t4}")
                            for ek in range(N_EB):
                                nc.tensor.matmul(
                                    pq[:], wqb[:, ek, :], xf_slice(t4, ek),
                                    start=(ek == 0), stop=(ek == N_EB - 1))
                            for ek in range(N_EB):
                                nc.tensor.matmul(
                                    pk[:], wkb[:, ek, :], xf_slice(t4, ek),
                                    start=(ek == 0), stop=(ek == N_EB - 1))
                            qs = pool_qs.tile([128, 512], DT, tag="qs",
                                              name=f"qs_{hm}_{t4}")
                            nc.scalar.copy(qs[:], pq[:])
                            nc.scalar.dma_start(
                                qT_dram[hm * 128:(hm + 1) * 128,
                                        t4 * 512:(t4 + 1) * 512], qs[:])
                            nc.vector.tensor_copy(
                                kt[hm][:, t4 * 512:(t4 + 1) * 512], pk[:])

            with (
                tc.tile_pool(name="v", bufs=1) as pool_v,
                tc.tile_pool(name="qpre", bufs=1) as pool_qpre,
            ):
                vt = [pool_v.tile([128, H], DT, tag=f"v{j}", name=f"v{j}")
                      for j in range(N_SB)]
                qc_pre = pool_qpre.tile([128, N_HB, TCB], DT, tag="qc_pre",
                                        name="qc_pre")
                nc.sync.dma_start(
                    qc_pre[:],
                    qT_dram[:, 0:TCB].rearrange("(k p) t -> p k t", p=128))

                # ---------------- phase A2: v (resident) -------------------
                with (
                    tc.tile_pool(name="x2", bufs=2) as pool_x2,
                    tc.tile_pool(name="pv", bufs=4, space="PSUM") as psum_v,
                ):
                    with nc.named_scope("proj_v"):
                        for t8 in range(T // 256):
                            if t8 == 0:
                                xv = xv_pre
                            else:
                                xv = pool_x2.tile([128, N_EB, 256], DT,
                                                  tag="xv", name=f"xv_{t8}")
                                nc.sync.dma_start(xv[:], xV_d[t8, :, :, :])
                            for ss in range(2):
                                j = t8 * 2 + ss
                                for hc in range(2):
                                    pv = psum_v.tile([128, 512], F32, tag="pv",
                                                     name=f"pv_{t8}_{ss}_{hc}")
                                    for ek in range(N_EB):
                                        nc.tensor.matmul(
                                            pv[:],
                                            xv[:, ek, ss * 128:(ss + 1) * 128],
                                            wvh[ek][hc][:],
                                            start=(ek == 0), stop=(ek == N_EB - 1))
                                    if hc == 0:
                                        nc.vector.tensor_copy(
                                            vt[j][:, hc * 512:(hc + 1) * 512], pv[:])
                                    else:
                                        nc.scalar.copy(
                                            vt[j][:, hc * 512:(hc + 1) * 512], pv[:])

                # ---------------- phase B: causal attention ----------------
                with (
                    tc.tile_pool(name="q", bufs=2) as pool_q,
                    tc.tile_pool(name="wt", bufs=3) as pool_wt,
                    tc.tile_pool(name="ob", bufs=4) as pool_ob,
                    tc.tile_pool(name="sm", bufs=4) as pool_sm,
                    tc.tile_pool(name="pb", bufs=1, space="PSUM") as psum_b,
                ):
                    with nc.named_scope("attn"):
                        for c in range(N_TCB):
                            if c == 0:
                                qc = qc_pre
                            else:
                                qc = pool_q.tile([128, N_HB, TCB], DT,
                                                 tag="qc", name=f"qc_{c}")
                                nc.sync.dma_start(
                                    qc[:],
                                    qT_dram[:, c * TCB:(c + 1) * TCB]
                                    .rearrange("(k p) t -> p k t", p=128))
                            n_j = 2 * c + 2
                            o_ps = [psum_b.tile([128, 512], F32, tag=f"O{i}",
                                                name=f"O_{c}_{i}")
                                    for i in range(4)]
                            rs_ps = psum_b.tile([1, TCB], F32, tag="rs",
                                                name=f"rs_{c}")

                            def scores(j, c=c, qc=qc):
                                s_ps = psum_b.tile([128, TCB], F32,
                                                   tag=f"S{j % 2}",
                                                   name=f"S_{c}_{j}")
                                for hk in range(N_HB):
                                    nc.tensor.matmul(
                                        s_ps[:],
                                        kt[hk][:, j * 128:(j + 1) * 128],
                                        qc[:, hk, :],
                                        start=(hk == 0), stop=(hk == N_HB - 1))
                                wt = pool_wt.tile([128, TCB], DT, tag="wt",
                                                  name=f"wt_{c}_{j}")
                                nc.scalar.activation(
                                    wt[:], s_ps[:],
                                    mybir.ActivationFunctionType.Exp,
                                    scale=SCALE)
                                if j >= 2 * c:
                                    # zero the strictly-upper (s > t) part
                                    nc.gpsimd.affine_select(
                                        out=wt[:], in_=wt[:],
                                        compare_op=mybir.AluOpType.is_ge,
                                        fill=0.0,
                                        base=c * TCB - j * 128,
                                        channel_multiplier=-1,
                                        pattern=[[1, TCB]])
                                return wt

                            def o_accum(j, wt, c=c, n_j=n_j, o_ps=o_ps,
                                        rs_ps=rs_ps):
                                first, last = (j == 0), (j == n_j - 1)
                                nc.tensor.matmul(
                                    rs_ps[:], ones[:], wt[:],
                                    start=first, stop=last)
                                for ts in range(2):
                                    if ts == 0 and j == n_j - 1 and n_j >= 2:
                                        # s-block 2c+1 is fully masked for
                                        # t-half 0: all-zero contribution
                                        continue
                                    wslice = wt[:, ts * 128:(ts + 1) * 128]
                                    last_ts = (j == n_j - 2) if ts == 0 \
                                        else last
                                    for hc in range(2):
                                        nc.tensor.matmul(
                                            o_ps[ts * 2 + hc][:], wslice,
                                            vt[j][:, hc * 512:(hc + 1) * 512],
                                            start=first, stop=last_ts)

                            # software pipeline: scores(j+1) issued ahead of
                            # O(j) so the PE never waits on the exp chain
                            wt_cur = scores(0)
                            for j in range(n_j):
                                wt_next = scores(j + 1) if j + 1 < n_j else None
                                o_accum(j, wt_cur)
                                wt_cur = wt_next
                            rs_sb = pool_sm.tile([1, TCB], F32, tag="rs_sb",
                                                 name=f"rs_sb_{c}")
                            nc.vector.tensor_copy(rs_sb[:], rs_ps[:])
                            for ts in range(2):
                                # transpose [1,128] -> [128,1] via K=1 matmul
                                rs_col = psum_b.tile([128, 1], F32,
                                                     tag="rs_col",
                                                     name=f"rs_col_{c}_{ts}")
                                nc.tensor.matmul(
                                    rs_col[:],
                                    rs_sb[0:1, ts * 128:(ts + 1) * 128],
                                    ones_f[0:1, 0:1],
                                    start=True, stop=True)
                                rec = pool_sm.tile([128, 1], F32, tag="rec",
                                                   name=f"rec_{c}_{ts}")
                                nc.vector.reciprocal(rec[:], rs_col[:])
                                for hc in range(2):
                                    ob = pool_ob.tile([128, 512], F32, tag="ob",
                                                      name=f"ob_{c}_{ts}_{hc}")
                                    if hc == 0:
                                        nc.vector.tensor_scalar_mul(
                                            ob[:], o_ps[ts * 2 + hc][:], rec[:])
                                    else:
                                        nc.scalar.activation(
                                            ob[:], o_ps[ts * 2 + hc][:],
                                            mybir.ActivationFunctionType.Copy,
                                            scale=rec[:])
                                    nc.gpsimd.dma_start(
                                        out_d[c * TCB + ts * 128:
                                              c * TCB + (ts + 1) * 128,
                                              hc * 512:(hc + 1) * 512], ob[:])

    nc.compile()
    return nc


_NC_CACHE = None


def _get_program():
    global _NC_CACHE
    if _NC_CACHE is None:
        _NC_CACHE = build_program()
    return _NC_CACHE


def make_in_maps(x, Wk, Wq, Wv):
    x = np.asarray(x, np.float32)
    xT = np.transpose(x, (0, 2, 1))                        # [B, E, T]
    # A1 layout [t4][p][ek][512]: xT[e, t] with e = ek*128 + p
    xA = np.ascontiguousarray(
        xT.reshape(B, N_EB, 128, 4, 512).transpose(0, 3, 2, 1, 4))
    # A2 layout [t8][p][ek][256]
    xV = np.ascontiguousarray(
        xT.reshape(B, N_EB, 128, 8, 256).transpose(0, 3, 2, 1, 4))

    def prep_w(W):   # [H,E] -> W.T [E,H] -> [hm][p][ek][128]
        WT = np.asarray(W, np.float32).T
        return np.ascontiguousarray(
            WT.reshape(N_EB, 128, N_HB, 128).transpose(2, 1, 0, 3))

    WqT = prep_w(Wq)
    WkT = prep_w(Wk)
    WvT = np.ascontiguousarray(np.asarray(Wv, np.float32).T)  # [E, H]
    return [{"xA": xA[b], "xV": xV[b], "WqT": WqT, "WkT": WkT, "WvT": WvT}
            for b in range(B)]


def kernel(x, Wk, Wq, Wv, _trace=False, _tmpdir=None):
    nc = _get_program()
    in_maps = make_in_maps(x, Wk, Wq, Wv)
    res = run_bass_kernel_spmd(nc, in_maps, list(range(N_CORES)),
                               trace=_trace, tmpdir=_tmpdir)
    out = np.stack([res.results[b]["out"] for b in range(B)])
    if _trace:
        kernel.last_result = res
    return out
